# revision 6
# baseline (speedup 1.0000x reference)
"""Block-diagonal masked dense + BatchNorm(train) + ReLU on 8 TRN2 NeuronCores.

Math: out = x @ (W * blockdiag_mask) + bias; BN over batch; relu.
The mask keeps 64 diagonal blocks of shape [64 in, 64 out]. Group g only
couples x[:, 64g:64g+64] to out[:, 64g:64g+64].

Sharding: groups are split across cores (8 groups per core). Each core owns a
disjoint 512-column slice of both input and output features, so the matmul and
the per-feature batch statistics are fully core-local (no collectives).

Per-core device program (all shapes hardcoded):
  inputs:  xT [512, 4096] bf16 (x slice transposed on host), wd [512, 128]
           bf16 (per 128-row chunk a 2x2 block-diagonal of two 64x64 group
           blocks), gm/bt [512] f32
  output:  yT [512, 4096] bf16 (host transposes back, fp32-casts)

Pipeline (the kernel is HBM-bound: 4.3 MB in + 4.2 MB out vs ~358 GB/s/core
=> ~24 us floor; everything else must hide under the DMA):
  - input streams chunk-major on BOTH HWDGE queues (sync + scalar) from t=0;
    scalar's issues are all up-front, before ACT has compute to do.
  - pass 1 per chunk: 8 matmuls K=128/N=512 -> PSUM, bn_stats (DVE) right
    behind each; bn_aggr + coefficient algebra; reciprocal is the only other
    DVE op, the rest of the coef chain rides ACT so DVE stays ~dense with
    stats (DVE is the pacing engine at ~22 us).
  - pass 2 per chunk: recompute the matmul (x stays SBUF-resident; PE has
    slack) and fold BN+relu into one ScalarE activation per 1024-col mega,
    PSUM -> SBUF bf16, store immediately (256 KB per store) on the gpsimd
    SWDGE queue so the output overlaps the remaining input.
  - last chunk: relu megas split ACT/DVE and stores split sync/gpsimd to
    shorten the serial drain at the end.

Accuracy: ~3e-3 rel L2 vs the fp32 reference (bf16 I/O rounding; BN math and
PSUM accumulation in fp32). bias never reaches the device: BN's mean
subtraction absorbs it exactly and variance is shift-invariant.
"""

import numpy as np

import concourse.bass as bass
import concourse.tile as tile
from concourse import mybir
from concourse.bass_utils import run_bass_kernel_spmd

F32 = mybir.dt.float32
BF16 = mybir.dt.bfloat16

NCORES = 8
BATCH = 4096
DIM = 4096
DCORE = DIM // NCORES          # 512 features per core
CHUNKS = DCORE // 128          # 4 partition chunks (2 groups each)
BTILE = 512                    # bn_stats tile (FMAX) / one PSUM bank
BTILES = BATCH // BTILE        # 8
MEGA = 1024                    # relu/store granularity (2 PSUM banks)
MEGAS = BATCH // MEGA          # 4
EPS = 1e-3

_MAX_WAITS = 1


def _split_multi_waits(nc: bass.Bass, max_waits: int = _MAX_WAITS) -> None:
    # The walrus build in this container rejects instructions carrying more
    # than one sync-wait command (any engine, any opcode). Hoist extra waits
    # onto same-engine NOPs inserted immediately before the instruction —
    # identical semantics, since the engine blocks on each wait in order.
    # Snapshot every block BEFORE creating any nop: the engine builders append
    # new instructions to the current (last) block as a side effect, and the
    # final wholesale reassignment below discards those spurious appends.
    snapshots = [
        (bb, list(bb.instructions)) for f in nc.m.functions for bb in f.blocks
    ]
    rebuilt = []
    for bb, insts in snapshots:
        new = []
        for ins in insts:
            si = getattr(ins, "sync_info", None)
            waits = list(si.on_wait) if si is not None and si.on_wait else []
            if len(waits) > max_waits:
                head = waits[:-max_waits]
                for i in range(0, len(head), max_waits):
                    nop = nc.engines[ins.engine].nop().ins
                    nop.sync_info = mybir.SyncInfo(
                        on_wait=head[i : i + max_waits], on_update=[]
                    )
                    new.append(nop)
                ins.sync_info = mybir.SyncInfo(
                    on_wait=waits[-max_waits:],
                    on_update=list(si.on_update or []),
                )
            new.append(ins)
        rebuilt.append((bb, new))
    for bb, new in rebuilt:
        bb.instructions = new


def _build_nc() -> bass.Bass:
    nc = bass.Bass()
    xT = nc.dram_tensor("xT", [DCORE, BATCH], BF16, kind="ExternalInput")
    # Partition-major [128, c*m]: one contiguous 1 KB descriptor per
    # partition (the [DCORE, 128] layout shattered into 512 x 256 B
    # descriptors and took ~3 us to land, stalling the first matmul).
    wd = nc.dram_tensor("wd", [128, CHUNKS * 128], BF16, kind="ExternalInput")
    # gamma/beta packed into one partition-major tensor: a single DMA.
    gb = nc.dram_tensor("gb", [128, 2 * CHUNKS], F32, kind="ExternalInput")
    yT = nc.dram_tensor("yT", [DCORE, BATCH], BF16, kind="ExternalOutput")

    with tile.TileContext(nc) as tc:
        with (
            tc.tile_pool(name="singles", bufs=1) as singles,
            tc.tile_pool(name="stats", bufs=1) as statp,
            tc.tile_pool(name="psum1", bufs=2, space="PSUM") as psum1,
            tc.tile_pool(name="psum2", bufs=3, space="PSUM") as psum2,
        ):
            xsb = singles.tile([128, CHUNKS, BATCH], BF16)
            xTv = xT.rearrange("(c p) b -> p c b", p=128)
            wsb = singles.tile([128, CHUNKS, 128], BF16)
            gbs = singles.tile([128, 2, CHUNKS], F32)
            gsb = gbs[:, 0, :]
            bsb = gbs[:, 1, :]
            zsb = singles.tile([128, CHUNKS, BATCH], BF16)
            yTv = yT.rearrange("(c p) b -> p c b", p=128)

            # Input plan. Every DMA costs ~0.65 us of issue time on its
            # engine plus ~1.5 us issue->data and ~1 us data->semaphore
            # latency, and a dependent op waits for the WHOLE transfer. So:
            # chunk-0 weights alone first (32 KB, unblocks LDWEIGHTS), then
            # chunk 0 in 128 KB tiles alternating queues (earliest possible
            # bn_stats start), coarser pieces for later chunks. scalar (=ACT)
            # issues all land before ACT has compute to do.
            wdv = wd.rearrange("p (c m) -> p c m", c=CHUNKS)
            nc.sync.dma_start(wsb[:, 0:1, :], wdv[:, 0:1, :])
            T = BTILE
            for t in range(4):
                eng = nc.sync if t % 2 == 0 else nc.scalar
                eng.dma_start(
                    xsb[:, 0, t * T : (t + 1) * T], xTv[:, 0, t * T : (t + 1) * T]
                )
            nc.scalar.dma_start(gbs[:], gb.rearrange("p (g c) -> p g c", g=2))
            nc.sync.dma_start(wsb[:, 1:, :], wdv[:, 1:, :])
            for t in range(4, 8):
                eng = nc.sync if t % 2 == 0 else nc.scalar
                eng.dma_start(
                    xsb[:, 0, t * T : (t + 1) * T], xTv[:, 0, t * T : (t + 1) * T]
                )
            Q = 1024
            for p in range(4):
                eng = nc.sync if p % 2 == 0 else nc.scalar
                eng.dma_start(
                    xsb[:, 1, p * Q : (p + 1) * Q], xTv[:, 1, p * Q : (p + 1) * Q]
                )
            H = BATCH // 2
            for c in range(2, CHUNKS):
                nc.sync.dma_start(xsb[:, c, :H], xTv[:, c, :H])
                nc.scalar.dma_start(xsb[:, c, H:], xTv[:, c, H:])

            epsb = singles.tile([128, 1], F32)
            nc.vector.memset(epsb[:], EPS)

            stats = statp.tile([128, CHUNKS, BTILES, 6], F32)
            mv = statp.tile([128, CHUNKS, 2], F32)
            coefA = statp.tile([128, CHUNKS], F32)
            coefB = statp.tile([128, CHUNKS], F32)
            tmp = statp.tile([128, CHUNKS], F32)

            def one_matmul(ps, os, c: int, t: int):
                # K=128 against a 2x2 block-diagonal stationary (two 64x64
                # group blocks; zeros kill the cross terms).
                nc.tensor.matmul(
                    ps[:, os],
                    lhsT=wsb[:, c, :],
                    rhs=xsb[:, c, bass.ds(t * BTILE, BTILE)],
                    start=True, stop=True,
                )

            def p1_tile(c: int, t: int):
                ps = psum1.tile([128, BTILE], F32, tag="ps1")
                one_matmul(ps, slice(None), c, t)
                nc.vector.bn_stats(stats[:, c, t, :], ps[:, :])

            def coef(c: int):
                # The DVE bn_stats chain is the kernel's critical path, so
                # it gets only two inserts per chunk: bn_aggr (DVE-only) and
                # one divide (A = gamma/sqrt; replaces reciprocal+multiply,
                # and Rsqrt/Reciprocal are banned on ACT). Sqrt and the B
                # algebra ride ACT in parallel with the next chunk's stats.
                nc.vector.bn_aggr(mv[:, c, :], stats[:, c, :, :])
                nc.scalar.activation(
                    tmp[:, c : c + 1], mv[:, c, 1:2],
                    mybir.ActivationFunctionType.Sqrt,
                    bias=epsb[:], scale=1.0,
                )
                nc.vector.reciprocal(tmp[:, c : c + 1], tmp[:, c : c + 1])
                # A = gamma * rsqrt(var+eps)  (divide is not a valid DVE
                # TensorTensor ALU op on this HW: s3s3d3_tt_valid_op)
                nc.vector.tensor_tensor(
                    coefA[:, c : c + 1], gsb[:, c : c + 1],
                    tmp[:, c : c + 1], mybir.AluOpType.mult,
                )
                # tmp = mean * A
                nc.scalar.activation(
                    tmp[:, c : c + 1], mv[:, c, 0:1],
                    mybir.ActivationFunctionType.Copy,
                    scale=coefA[:, c : c + 1],
                )
                # B = beta - mean * A  (Identity: out = in*scale + bias)
                nc.scalar.activation(
                    coefB[:, c : c + 1], tmp[:, c : c + 1],
                    mybir.ActivationFunctionType.Identity,
                    bias=bsb[:, c : c + 1], scale=-1.0,
                )

            def p2_mega(c: int, m: int):
                # Recompute the matmul (x stays SBUF-resident) and fold
                # BN+relu into one pass, PSUM -> SBUF bf16 -> DRAM. The last
                # chunk's megas alternate ACT/DVE and sync/gpsimd stores to
                # shorten the final serial drain.
                ps = psum2.tile([128, MEGA], F32, tag="ps2")
                for q in range(MEGA // BTILE):
                    one_matmul(
                        ps, bass.ds(q * BTILE, BTILE), c,
                        m * (MEGA // BTILE) + q,
                    )
                msl = bass.ds(m * MEGA, MEGA)
                last = c == CHUNKS - 1
                if last and m % 2 == 1:
                    # z = relu(A*y + B) on DVE: affine (PSUM src, 1x) then
                    # max(0) at 4x. Frees ACT for the other megas.
                    nc.vector.tensor_scalar(
                        zsb[:, c, msl], ps[:],
                        coefA[:, c : c + 1], coefB[:, c : c + 1],
                        mybir.AluOpType.mult, mybir.AluOpType.add,
                    )
                    nc.vector.tensor_scalar(
                        zsb[:, c, msl], zsb[:, c, msl],
                        0.0, None, mybir.AluOpType.max,
                    )
                else:
                    nc.scalar.activation(
                        zsb[:, c, msl], ps[:],
                        mybir.ActivationFunctionType.Relu,
                        bias=coefB[:, c : c + 1], scale=coefA[:, c : c + 1],
                    )
                eng = nc.sync if (last and m % 2 == 0) else nc.gpsimd
                eng.dma_start(yTv[:, c, msl], zsb[:, c, msl])

            for t in range(BTILES):
                p1_tile(0, t)
            coef(0)
            for c in range(CHUNKS):
                for m in range(MEGAS):
                    p2_mega(c, m)
                    if c + 1 < CHUNKS:
                        p1_tile(c + 1, 2 * m)
                        p1_tile(c + 1, 2 * m + 1)
                if c + 1 < CHUNKS:
                    coef(c + 1)
    _split_multi_waits(nc)
    return nc


_NC_CACHE: bass.Bass | None = None


def _get_nc() -> bass.Bass:
    global _NC_CACHE
    if _NC_CACHE is None:
        _NC_CACHE = _build_nc()
    return _NC_CACHE


from ml_dtypes import bfloat16 as _bf16


def _make_in_maps(x, weight, gamma, beta):
    in_maps = []
    for c in range(NCORES):
        sl = slice(c * DCORE, (c + 1) * DCORE)
        xT = np.ascontiguousarray(x[:, sl].T).astype(_bf16)
        # Per 128-row chunk: [[w_{2c}, 0], [0, w_{2c+1}]] block-diagonal.
        wdc = np.zeros((DCORE, 128), np.float32)
        for g in range(DCORE // 64):
            r = slice(c * DCORE + g * 64, c * DCORE + (g + 1) * 64)
            col = (g % 2) * 64
            wdc[g * 64 : (g + 1) * 64, col : col + 64] = weight[r, r]
        # Partition-major: wd2[p, 128c+m] = wdc[128c+p, m]; 1 KB contiguous
        # per partition so the weight DMA is one descriptor per partition.
        wd2 = np.ascontiguousarray(
            wdc.reshape(CHUNKS, 128, 128).transpose(1, 0, 2).reshape(128, -1)
        )
        g2 = np.ascontiguousarray(gamma[sl].reshape(CHUNKS, 128).T)
        b2 = np.ascontiguousarray(beta[sl].reshape(CHUNKS, 128).T)
        in_maps.append(
            {
                "xT": xT,
                "wd": wd2.astype(_bf16),
                "gb": np.concatenate([g2, b2], axis=1).astype(np.float32),
            }
        )
    return in_maps


def kernel(x, weight, bias, gamma, beta, **_run_kwargs) -> np.ndarray:
    x = np.asarray(x, np.float32)
    weight = np.asarray(weight, np.float32)
    gamma = np.asarray(gamma, np.float32)
    beta = np.asarray(beta, np.float32)
    # bias is algebraically irrelevant: BN subtracts the batch mean, which
    # absorbs any constant per-feature shift, and variance is shift-invariant.

    nc = _get_nc()
    res = run_bass_kernel_spmd(
        nc, _make_in_maps(x, weight, gamma, beta),
        core_ids=list(range(NCORES)), **_run_kwargs,
    )
    out = np.empty((BATCH, DIM), np.float32)
    for c, r in enumerate(res.results):
        out[:, c * DCORE : (c + 1) * DCORE] = r["yT"].T.astype(np.float32)
    kernel.last_results = res
    return out


# revision 7
# speedup vs baseline: 1.0048x; 1.0048x over previous
"""Block-diagonal masked dense + BatchNorm(train) + ReLU on 8 TRN2 NeuronCores.

Math: out = x @ (W * blockdiag_mask) + bias; BN over batch; relu.
The mask keeps 64 diagonal blocks of shape [64 in, 64 out]. Group g only
couples x[:, 64g:64g+64] to out[:, 64g:64g+64].

Sharding: groups are split across cores (8 groups per core). Each core owns a
disjoint 512-column slice of both input and output features, so the matmul and
the per-feature batch statistics are fully core-local (no collectives).

Per-core device program (all shapes hardcoded):
  inputs:  xT [512, 4096] bf16 (x slice transposed on host), wd [512, 128]
           bf16 (per 128-row chunk a 2x2 block-diagonal of two 64x64 group
           blocks), gm/bt [512] f32
  output:  yT [512, 4096] bf16 (host transposes back, fp32-casts)

Pipeline (the kernel is HBM-bound: 4.3 MB in + 4.2 MB out vs ~358 GB/s/core
=> ~24 us floor; everything else must hide under the DMA):
  - input streams chunk-major on BOTH HWDGE queues (sync + scalar) from t=0;
    scalar's issues are all up-front, before ACT has compute to do.
  - pass 1 per chunk: 8 matmuls K=128/N=512 -> PSUM, bn_stats (DVE) right
    behind each; bn_aggr + coefficient algebra; reciprocal is the only other
    DVE op, the rest of the coef chain rides ACT so DVE stays ~dense with
    stats (DVE is the pacing engine at ~22 us).
  - pass 2 per chunk: recompute the matmul (x stays SBUF-resident; PE has
    slack) and fold BN+relu into one ScalarE activation per 1024-col mega,
    PSUM -> SBUF bf16, store immediately (256 KB per store) on the gpsimd
    SWDGE queue so the output overlaps the remaining input.
  - last chunk: relu megas split ACT/DVE and stores split sync/gpsimd to
    shorten the serial drain at the end.

Accuracy: ~3e-3 rel L2 vs the fp32 reference (bf16 I/O rounding; BN math and
PSUM accumulation in fp32). bias never reaches the device: BN's mean
subtraction absorbs it exactly and variance is shift-invariant.
"""

import numpy as np

import concourse.bass as bass
import concourse.tile as tile
from concourse import mybir
from concourse.bass_utils import run_bass_kernel_spmd

F32 = mybir.dt.float32
BF16 = mybir.dt.bfloat16

NCORES = 8
BATCH = 4096
DIM = 4096
DCORE = DIM // NCORES          # 512 features per core
CHUNKS = DCORE // 128          # 4 partition chunks (2 groups each)
BTILE = 512                    # bn_stats tile (FMAX) / one PSUM bank
BTILES = BATCH // BTILE        # 8
MEGA = 1024                    # relu/store granularity (2 PSUM banks)
MEGAS = BATCH // MEGA          # 4
EPS = 1e-3

_MAX_WAITS = 1


def _split_multi_waits(nc: bass.Bass, max_waits: int = _MAX_WAITS) -> None:
    # The walrus build in this container rejects instructions carrying more
    # than one sync-wait command (any engine, any opcode). Hoist extra waits
    # onto same-engine NOPs inserted immediately before the instruction —
    # identical semantics, since the engine blocks on each wait in order.
    # Snapshot every block BEFORE creating any nop: the engine builders append
    # new instructions to the current (last) block as a side effect, and the
    # final wholesale reassignment below discards those spurious appends.
    snapshots = [
        (bb, list(bb.instructions)) for f in nc.m.functions for bb in f.blocks
    ]
    rebuilt = []
    for bb, insts in snapshots:
        new = []
        for ins in insts:
            si = getattr(ins, "sync_info", None)
            waits = list(si.on_wait) if si is not None and si.on_wait else []
            if len(waits) > max_waits:
                head = waits[:-max_waits]
                for i in range(0, len(head), max_waits):
                    nop = nc.engines[ins.engine].nop().ins
                    nop.sync_info = mybir.SyncInfo(
                        on_wait=head[i : i + max_waits], on_update=[]
                    )
                    new.append(nop)
                ins.sync_info = mybir.SyncInfo(
                    on_wait=waits[-max_waits:],
                    on_update=list(si.on_update or []),
                )
            new.append(ins)
        rebuilt.append((bb, new))
    for bb, new in rebuilt:
        bb.instructions = new


def _build_nc() -> bass.Bass:
    nc = bass.Bass()
    xT = nc.dram_tensor("xT", [DCORE, BATCH], BF16, kind="ExternalInput")
    # Partition-major [128, c*m]: one contiguous 1 KB descriptor per
    # partition (the [DCORE, 128] layout shattered into 512 x 256 B
    # descriptors and took ~3 us to land, stalling the first matmul).
    wd = nc.dram_tensor("wd", [128, CHUNKS * 128], BF16, kind="ExternalInput")
    # gamma/beta packed into one partition-major tensor: a single DMA.
    gb = nc.dram_tensor("gb", [128, 2 * CHUNKS], F32, kind="ExternalInput")
    yT = nc.dram_tensor("yT", [DCORE, BATCH], BF16, kind="ExternalOutput")

    with tile.TileContext(nc) as tc:
        with (
            tc.tile_pool(name="singles", bufs=1) as singles,
            tc.tile_pool(name="stats", bufs=1) as statp,
            tc.tile_pool(name="psum1", bufs=4, space="PSUM") as psum1,
            tc.tile_pool(name="psum2", bufs=2, space="PSUM") as psum2,
        ):
            xsb = singles.tile([128, CHUNKS, BATCH], BF16)
            xTv = xT.rearrange("(c p) b -> p c b", p=128)
            wsb = singles.tile([128, CHUNKS, 128], BF16)
            gbs = singles.tile([128, 2, CHUNKS], F32)
            gsb = gbs[:, 0, :]
            bsb = gbs[:, 1, :]
            zsb = singles.tile([128, CHUNKS, BATCH], BF16)
            yTv = yT.rearrange("(c p) b -> p c b", p=128)

            # Input plan. Every DMA costs ~0.65 us of issue time on its
            # engine plus ~1.5 us issue->data and ~1 us data->semaphore
            # latency, and a dependent op waits for the WHOLE transfer. So:
            # chunk-0 weights alone first (32 KB, unblocks LDWEIGHTS), then
            # chunk 0 in 128 KB tiles alternating queues (earliest possible
            # bn_stats start), coarser pieces for later chunks. scalar (=ACT)
            # issues all land before ACT has compute to do.
            wdv = wd.rearrange("p (c m) -> p c m", c=CHUNKS)
            nc.sync.dma_start(wsb[:, 0:1, :], wdv[:, 0:1, :])
            T = BTILE
            for t in range(4):
                eng = nc.sync if t % 2 == 0 else nc.scalar
                eng.dma_start(
                    xsb[:, 0, t * T : (t + 1) * T], xTv[:, 0, t * T : (t + 1) * T]
                )
            nc.scalar.dma_start(gbs[:], gb.rearrange("p (g c) -> p g c", g=2))
            nc.sync.dma_start(wsb[:, 1:, :], wdv[:, 1:, :])
            nc.sync.dma_start(xsb[:, 0, 4 * T : 6 * T], xTv[:, 0, 4 * T : 6 * T])
            nc.scalar.dma_start(xsb[:, 0, 6 * T :], xTv[:, 0, 6 * T :])
            H = BATCH // 2
            for c in range(1, CHUNKS):
                nc.sync.dma_start(xsb[:, c, :H], xTv[:, c, :H])
                nc.scalar.dma_start(xsb[:, c, H:], xTv[:, c, H:])

            epsb = singles.tile([128, 1], F32)
            nc.vector.memset(epsb[:], EPS)

            stats = statp.tile([128, CHUNKS, BTILES, 6], F32)
            mv = statp.tile([128, CHUNKS, 2], F32)
            coefA = statp.tile([128, CHUNKS], F32)
            coefB = statp.tile([128, CHUNKS], F32)
            tmp = statp.tile([128, CHUNKS], F32)

            def one_matmul(ps, os, c: int, t: int):
                # K=128 against a 2x2 block-diagonal stationary (two 64x64
                # group blocks; zeros kill the cross terms).
                nc.tensor.matmul(
                    ps[:, os],
                    lhsT=wsb[:, c, :],
                    rhs=xsb[:, c, bass.ds(t * BTILE, BTILE)],
                    start=True, stop=True,
                )

            def p1_tile(c: int, t: int):
                ps = psum1.tile([128, BTILE], F32, tag="ps1")
                one_matmul(ps, slice(None), c, t)
                nc.vector.bn_stats(stats[:, c, t, :], ps[:, :])

            def coef(c: int):
                # The DVE bn_stats chain is the kernel's critical path, so
                # it gets only two inserts per chunk: bn_aggr (DVE-only) and
                # one divide (A = gamma/sqrt; replaces reciprocal+multiply,
                # and Rsqrt/Reciprocal are banned on ACT). Sqrt and the B
                # algebra ride ACT in parallel with the next chunk's stats.
                nc.vector.bn_aggr(mv[:, c, :], stats[:, c, :, :])
                nc.scalar.activation(
                    tmp[:, c : c + 1], mv[:, c, 1:2],
                    mybir.ActivationFunctionType.Sqrt,
                    bias=epsb[:], scale=1.0,
                )
                nc.vector.reciprocal(tmp[:, c : c + 1], tmp[:, c : c + 1])
                # A = gamma * rsqrt(var+eps)  (divide is not a valid DVE
                # TensorTensor ALU op on this HW: s3s3d3_tt_valid_op)
                nc.vector.tensor_tensor(
                    coefA[:, c : c + 1], gsb[:, c : c + 1],
                    tmp[:, c : c + 1], mybir.AluOpType.mult,
                )
                # tmp = mean * A
                nc.scalar.activation(
                    tmp[:, c : c + 1], mv[:, c, 0:1],
                    mybir.ActivationFunctionType.Copy,
                    scale=coefA[:, c : c + 1],
                )
                # B = beta - mean * A  (Identity: out = in*scale + bias)
                nc.scalar.activation(
                    coefB[:, c : c + 1], tmp[:, c : c + 1],
                    mybir.ActivationFunctionType.Identity,
                    bias=bsb[:, c : c + 1], scale=-1.0,
                )

            def p2_mega(c: int, m: int):
                # Recompute the matmul (x stays SBUF-resident) and fold
                # BN+relu into one pass, PSUM -> SBUF bf16 -> DRAM. The last
                # chunk's megas alternate ACT/DVE and sync/gpsimd stores to
                # shorten the final serial drain.
                ps = psum2.tile([128, MEGA], F32, tag="ps2")
                for q in range(MEGA // BTILE):
                    one_matmul(
                        ps, bass.ds(q * BTILE, BTILE), c,
                        m * (MEGA // BTILE) + q,
                    )
                msl = bass.ds(m * MEGA, MEGA)
                last = c == CHUNKS - 1
                if last and m % 2 == 1:
                    # z = relu(A*y + B) on DVE: affine (PSUM src, 1x) then
                    # max(0) at 4x. Frees ACT for the other megas.
                    nc.vector.tensor_scalar(
                        zsb[:, c, msl], ps[:],
                        coefA[:, c : c + 1], coefB[:, c : c + 1],
                        mybir.AluOpType.mult, mybir.AluOpType.add,
                    )
                    nc.vector.tensor_scalar(
                        zsb[:, c, msl], zsb[:, c, msl],
                        0.0, None, mybir.AluOpType.max,
                    )
                else:
                    nc.scalar.activation(
                        zsb[:, c, msl], ps[:],
                        mybir.ActivationFunctionType.Relu,
                        bias=coefB[:, c : c + 1], scale=coefA[:, c : c + 1],
                    )
                eng = nc.sync if (last and m % 2 == 0) else nc.gpsimd
                eng.dma_start(yTv[:, c, msl], zsb[:, c, msl])

            for t in range(BTILES):
                p1_tile(0, t)
            coef(0)
            for c in range(CHUNKS):
                for m in range(MEGAS):
                    p2_mega(c, m)
                    if c + 1 < CHUNKS:
                        p1_tile(c + 1, 2 * m)
                        p1_tile(c + 1, 2 * m + 1)
                if c + 1 < CHUNKS:
                    coef(c + 1)
    _split_multi_waits(nc)
    return nc


_NC_CACHE: bass.Bass | None = None


def _get_nc() -> bass.Bass:
    global _NC_CACHE
    if _NC_CACHE is None:
        _NC_CACHE = _build_nc()
    return _NC_CACHE


from ml_dtypes import bfloat16 as _bf16


def _make_in_maps(x, weight, gamma, beta):
    in_maps = []
    for c in range(NCORES):
        sl = slice(c * DCORE, (c + 1) * DCORE)
        xT = np.ascontiguousarray(x[:, sl].T).astype(_bf16)
        # Per 128-row chunk: [[w_{2c}, 0], [0, w_{2c+1}]] block-diagonal.
        wdc = np.zeros((DCORE, 128), np.float32)
        for g in range(DCORE // 64):
            r = slice(c * DCORE + g * 64, c * DCORE + (g + 1) * 64)
            col = (g % 2) * 64
            wdc[g * 64 : (g + 1) * 64, col : col + 64] = weight[r, r]
        # Partition-major: wd2[p, 128c+m] = wdc[128c+p, m]; 1 KB contiguous
        # per partition so the weight DMA is one descriptor per partition.
        wd2 = np.ascontiguousarray(
            wdc.reshape(CHUNKS, 128, 128).transpose(1, 0, 2).reshape(128, -1)
        )
        g2 = np.ascontiguousarray(gamma[sl].reshape(CHUNKS, 128).T)
        b2 = np.ascontiguousarray(beta[sl].reshape(CHUNKS, 128).T)
        in_maps.append(
            {
                "xT": xT,
                "wd": wd2.astype(_bf16),
                "gb": np.concatenate([g2, b2], axis=1).astype(np.float32),
            }
        )
    return in_maps


def kernel(x, weight, bias, gamma, beta, **_run_kwargs) -> np.ndarray:
    x = np.asarray(x, np.float32)
    weight = np.asarray(weight, np.float32)
    gamma = np.asarray(gamma, np.float32)
    beta = np.asarray(beta, np.float32)
    # bias is algebraically irrelevant: BN subtracts the batch mean, which
    # absorbs any constant per-feature shift, and variance is shift-invariant.

    nc = _get_nc()
    res = run_bass_kernel_spmd(
        nc, _make_in_maps(x, weight, gamma, beta),
        core_ids=list(range(NCORES)), **_run_kwargs,
    )
    out = np.empty((BATCH, DIM), np.float32)
    for c, r in enumerate(res.results):
        out[:, c * DCORE : (c + 1) * DCORE] = r["yT"].T.astype(np.float32)
    kernel.last_results = res
    return out
